# revision 15
# baseline (speedup 1.0000x reference)
"""Trainium2 Bass kernel for the per-feature grouped MLP (SuperLinear/GLU x2).

Math (per feature d of D=2048, batch B=512, M=32, H=64):
  x1 = state[:, d, :] @ w1a[:, :, d] / Ta + b1a[d]      [B, 128]
  h  = x1[:, :64] * sigmoid(x1[:, 64:])                 [B, 64]
  x2 = h @ w1b[:, :, d] / Tb + b1b[d]                   [B, 2]
  out[:, d] = x2[:, 0] * sigmoid(x2[:, 1])

Sharding: D split across 8 cores (embarrassingly parallel), 256 features/core.

Device dataflow per core (gen = 4 features, super-gen = 2 gens):
  MM1 (fp16): PE in 32x64 array-tiling mode. Feature j of a gen owns K-rows
  32j..32j+32. Even gens map feature j to array col-half (j%2), odd gens to
  (1-j%2), so the two gens of a super-gen use complementary array tiles and
  run concurrently (8 tiles busy). Bank layout: A = [a_even; a_odd] per
  feature-pair, G likewise => GLU operands are partition-aligned. A/G are
  [128, 1024] (2 PSUM banks, one pair per bank).
  GLU1: sigmoid (ACT) [128,1024] PSUM->SBUF fp32; tensor_mul (DVE)
  [128,1024] -> x2 (fp16, SBUF). For odd gens the pair rows are swapped
  (f_odd on top) — compensated in the host-built w2 quad weights.
  MM2 (fp16): x2 chunk [128,128] stationary (fp16 => FWL), rhs = block-diag
  pair weights [128, 4] -> quad columns [128b, 4] into a per-window PSUM
  bank at col bc*QR + pair_local*4. Burst every `burst` gens.
  GLU2 per 16-gen window: strided sigmoid over c1 cols + tensor_mul, one
  3D-AP output DMA.
"""

import numpy as np

_CACHE = {}


def _build_nc(B, DL, M, H, burst, window, use_ba, use_bg, use_bq):
    import concourse.bass as bass
    import concourse.mybir as mybir
    from concourse import bacc
    from concourse.tile import TileContext

    f32 = mybir.dt.float32
    f16 = mybir.dt.float16
    H2 = 2 * H
    NGEN = DL // 4  # gens of 4 features
    assert NGEN % window == 0 and window % burst == 0
    assert NGEN % 2 == 0 and burst % 2 == 0
    NB = B // 128  # b-chunks for MM2
    QR = 8 * window  # quad cols per b-chunk region (2*window pairs x 4)
    FW = 4 * window  # features (output cols) per window

    nc = bacc.Bacc("TRN2", target_bir_lowering=False)

    # st: [128=(j,m), NGEN*B]; w: [128=(j,m), NGEN*128=(gen,(wa|wg))]
    st_d = nc.dram_tensor("st", [128, NGEN * B], f16, kind="ExternalInput")
    w_d = nc.dram_tensor("w", [128, NGEN * H2], f16, kind="ExternalInput")
    # w2 quad weights, window-major: [nwin*128, 32*4]
    w2_d = nc.dram_tensor("w2", [(NGEN // window) * H2, 2 * window * 4], f16,
                          kind="ExternalInput")
    if use_bg:
        bg_d = nc.dram_tensor("bg", [DL, H], f32, kind="ExternalInput")
    if use_ba:
        ba_d = nc.dram_tensor("ba", [DL, H], f32, kind="ExternalInput")
    if use_bq:
        bq_d = nc.dram_tensor("bq", [DL // 2, 4], f32, kind="ExternalInput")
    out_d = nc.dram_tensor("out", [B, DL], f32, kind="ExternalOutput")

    Sig = mybir.ActivationFunctionType.Sigmoid
    Mult = mybir.AluOpType.mult

    with TileContext(nc) as tc:
        with tc.tile_pool(name="sb", bufs=4) as sb:
            x2_list = {}
            if use_bq:
                bq_t = sb.tile([1, DL * 2], f32, tag="bq", bufs=1, name="bqt")
                ones_t = sb.tile([1, 128], f16, tag="ones", bufs=1,
                                 name="onest")
                nc.sync.dma_start(out=bq_t,
                                  in_=bq_d.rearrange("p q -> 1 (p q)"))
                nc.vector.memset(ones_t, 1.0)

            # ================= Phase 1: MM1 + GLU1, x2 staged ==========
            with tc.tile_pool(name="ps1", bufs=4, space="PSUM") as ps:
                for sg in range(NGEN // 2):
                    g0 = 2 * sg
                    st_t = sb.tile([128, 2 * B], f16, tag="st", bufs=6,
                                   name=f"st{sg}")
                    nc.sync.dma_start(out=st_t,
                                      in_=st_d[:, g0 * B:(g0 + 2) * B])
                    w_t = sb.tile([128, 2 * H2], f16, tag="w", bufs=6,
                                  name=f"w{sg}")
                    nc.gpsimd.dma_start(out=w_t,
                                        in_=w_d[:, g0 * H2:(g0 + 2) * H2])

                    gens = []
                    for gi in range(2):
                        g = g0 + gi
                        # Separate tags decouple the sigmoid slot-recycle
                        # loop from the (slower) multiply loop.
                        G = ps.tile([128, 2 * B], f32, tag="gb", bufs=2,
                                    name=f"G{g}")
                        A = ps.tile([128, 2 * B], f32, tag="ab", bufs=2,
                                    name=f"A{g}")
                        gens.append((g, gi, A, G))

                    # MM1: 16 tiled matmuls, 2 rounds x 8 concurrent.
                    # gen parity gi: feature j -> array col-half (j%2)^gi.
                    for rnd in range(2):
                        for g, gi, A, G in gens:
                            for j in range(4):
                                rs = slice(32 * j, 32 * j + 32)
                                cp = 64 * ((j % 2) ^ gi)
                                fb = B * (j // 2)
                                do_a = (j % 2) == rnd
                                dst = A if do_a else G
                                wq = 0 if do_a else H
                                nc.tensor.matmul(
                                    out=dst[cp:cp + 64, fb:fb + B],
                                    lhsT=w_t[rs,
                                             gi * H2 + wq:gi * H2 + wq + H],
                                    rhs=st_t[rs, gi * B:(gi + 1) * B],
                                    start=True, stop=True,
                                    tile_position=(32 * j, cp))

                    # GLU1 per gen
                    for g, gi, A, G in gens:
                        sig = sb.tile([128, 2 * B], f32, tag="sig",
                                      name=f"sig{g}")
                        x2 = sb.tile([128, 2 * B], f16, tag="x2", bufs=NGEN,
                                     name=f"x2{g}")
                        if use_bg:
                            bg_t = sb.tile([128, 2], f32, tag="bg",
                                           name=f"bg{g}")
                            nc.sync.dma_start(
                                out=bg_t,
                                in_=bg_d[4 * g:4 * g + 4, :].rearrange(
                                    "(p two) h -> (two h) p", two=2))
                            for k in range(2):
                                nc.scalar.activation(
                                    out=sig[:, k * B:(k + 1) * B],
                                    in_=G[:, k * B:(k + 1) * B], func=Sig,
                                    bias=bg_t[:, k:k + 1])
                        else:
                            nc.scalar.activation(out=sig, in_=G, func=Sig)
                        if use_ba:
                            ba_t = sb.tile([128, 2], f32, tag="ba",
                                           name=f"ba{g}")
                            nc.sync.dma_start(
                                out=ba_t,
                                in_=ba_d[4 * g:4 * g + 4, :].rearrange(
                                    "(p two) h -> (two h) p", two=2))
                            for k in range(2):
                                nc.vector.scalar_tensor_tensor(
                                    out=x2[:, k * B:(k + 1) * B],
                                    in0=A[:, k * B:(k + 1) * B],
                                    scalar=ba_t[:, k:k + 1],
                                    in1=sig[:, k * B:(k + 1) * B],
                                    op0=mybir.AluOpType.add, op1=Mult)
                        else:
                            nc.vector.tensor_tensor(out=x2, in0=A, in1=sig,
                                                    op=Mult)
                        x2_list[g] = x2

            # ================= Phase 2: MM2 + GLU2 =====================
            with tc.tile_pool(name="ps2", bufs=4, space="PSUM") as ps2:
                for w in range(NGEN // window):
                    quad = ps2.tile([128, NB * QR], f32, tag="quad", bufs=4,
                                    name=f"quad{w}")
                    w2_t = sb.tile([H2, 2 * window * 4], f16, tag="w2",
                                   bufs=2, name=f"w2t{w}")
                    nc.gpsimd.dma_start(
                        out=w2_t, in_=w2_d[w * H2:(w + 1) * H2, :])
                    for gg in range(w * window, (w + 1) * window):
                        x2g = x2_list.pop(gg)
                        for p in range(2):
                            pl = (gg % window) * 2 + p  # pair idx in window
                            for bc in range(NB):
                                qo = bc * QR + pl * 4
                                nc.tensor.matmul(
                                    out=quad[:, qo:qo + 4],
                                    lhsT=x2g[:, p * B + bc * 128:
                                             p * B + (bc + 1) * 128],
                                    rhs=w2_t[:, pl * 4:pl * 4 + 4],
                                    start=True, stop=not use_bq)
                                if use_bq:
                                    pg = (gg * 2 + p)  # global pair
                                    nc.tensor.matmul(
                                        out=quad[:, qo:qo + 4],
                                        lhsT=ones_t,
                                        rhs=bq_t[:, pg * 4:pg * 4 + 4],
                                        start=False, stop=True)
                    sig2 = sb.tile([128, NB * QR // 2], f32, tag="sig2",
                                   bufs=2, name=f"sig2{w}")
                    o_t = sb.tile([128, NB * QR // 2], f32, tag="o", bufs=2,
                                  name=f"o{w}")
                    nc.scalar.activation(
                        out=sig2, in_=quad[:, 1:NB * QR:2], func=Sig)
                    nc.vector.tensor_tensor(
                        out=o_t, in0=quad[:, 0:NB * QR:2], in1=sig2, op=Mult)
                    dst = out_d.rearrange("(bc p) d -> p bc d", bc=NB)
                    nc.sync.dma_start(
                        out=dst[:, :, w * FW:(w + 1) * FW],
                        in_=o_t.rearrange("p (bc d) -> p bc d", bc=NB))
    nc.finalize()
    return nc


def _gen_major(a, NC, NGEN):
    """[D, 32, X] -> per-core [128=(j,m), NGEN*X] with gen-major free dim."""
    D = a.shape[0]
    X = a.shape[2]
    r = a.reshape(NC, NGEN, 4 * 32, X).transpose(0, 2, 1, 3)
    return np.ascontiguousarray(r.reshape(NC, 128, NGEN * X))


def _host_prep(state_trace, w1a, b1a, Ta, w1b, b1b, Tb, NC):
    import ml_dtypes  # noqa: F401  (fp16 is native numpy)

    B, D, M = state_trace.shape
    H2 = w1a.shape[1]
    H = H2 // 2
    DL = D // NC
    window = 16
    NGEN = DL // 4

    Ta_v = float(np.asarray(Ta).reshape(-1)[0])
    Tb_v = float(np.asarray(Tb).reshape(-1)[0])

    # state: [B, D, M] -> [D, M, B] fp16 -> gen-major
    st = np.asarray(state_trace, np.float32).transpose(1, 2, 0)
    st = _gen_major(st.astype(np.float16), NC, NGEN)

    # w1a: [M, 2H, D]/Ta -> [D, M, 2H] fp16 (cols: wa | wg) -> gen-major
    w1aT = (np.asarray(w1a, np.float32).transpose(2, 0, 1)
            * np.float32(1.0 / Ta_v))
    w = _gen_major(w1aT.astype(np.float16), NC, NGEN)

    # w2 block-diag quads: [D/2 pairs, 2H, 4], cols (c0f0,c1f0,c0f1,c1f1)
    # where f0 = even feature of the pair. For pairs of ODD gens the x2
    # partition blocks are swapped (f_odd on top), so swap the row blocks.
    w1bT = (np.asarray(w1b, np.float32).transpose(2, 0, 1)
            * np.float32(1.0 / Tb_v))  # [D, H, 2]
    w2q = np.zeros((D // 2, H2, 4), np.float32)
    pr = np.arange(D // 2)
    odd = (pr // 2) % 2 == 1  # pair's gen parity
    ev = ~odd
    w2q[ev, :H, 0] = w1bT[0::2][ev, :, 0]
    w2q[ev, :H, 1] = w1bT[0::2][ev, :, 1]
    w2q[ev, H:, 2] = w1bT[1::2][ev, :, 0]
    w2q[ev, H:, 3] = w1bT[1::2][ev, :, 1]
    w2q[odd, H:, 0] = w1bT[0::2][odd, :, 0]
    w2q[odd, H:, 1] = w1bT[0::2][odd, :, 1]
    w2q[odd, :H, 2] = w1bT[1::2][odd, :, 0]
    w2q[odd, :H, 3] = w1bT[1::2][odd, :, 1]
    nwin = NGEN // window
    w2q = w2q.reshape(NC, nwin, 2 * window, H2, 4).transpose(0, 1, 3, 2, 4)
    w2q = np.ascontiguousarray(
        w2q.reshape(NC, nwin * H2, 2 * window * 4)).astype(np.float16)

    # biases (device order: for odd gens the pair rows are swapped)
    b1a_f = np.asarray(b1a, np.float32).reshape(D, H2) * np.float32(1 / Ta_v)
    gperm = np.arange(D).reshape(-1, 4)
    gperm[1::2] = gperm[1::2][:, [1, 0, 3, 2]]
    gperm = gperm.reshape(-1)
    ba = np.ascontiguousarray(b1a_f[gperm, :H])
    bg = np.ascontiguousarray(b1a_f[gperm, H:])
    b1b_f = np.asarray(b1b, np.float32).reshape(D, 2) * np.float32(1 / Tb_v)
    bq = np.zeros((D // 2, 4), np.float32)
    bq[:, 0] = b1b_f[0::2, 0]
    bq[:, 1] = b1b_f[0::2, 1]
    bq[:, 2] = b1b_f[1::2, 0]
    bq[:, 3] = b1b_f[1::2, 1]

    use_ba = bool(np.any(ba))
    use_bg = bool(np.any(bg))
    use_bq = bool(np.any(bq))

    in_maps = []
    for c in range(NC):
        ds = slice(c * DL, (c + 1) * DL)
        m = {"st": st[c], "w": w[c], "w2": w2q[c]}
        if use_bg:
            m["bg"] = np.ascontiguousarray(bg[ds])
        if use_ba:
            m["ba"] = np.ascontiguousarray(ba[ds])
        if use_bq:
            m["bq"] = np.ascontiguousarray(bq[c * DL // 2:(c + 1) * DL // 2])
        in_maps.append(m)
    import os
    burst = int(os.environ.get("K_BURST", "4"))
    cfg = dict(B=B, DL=DL, M=M, H=H, burst=burst, window=window,
               use_ba=use_ba, use_bg=use_bg, use_bq=use_bq)
    return in_maps, cfg


def kernel(state_trace, w1a, b1a, Ta, w1b, b1b, Tb):
    from concourse.bass_utils import run_bass_kernel_spmd

    NC = 8
    B, D, M = state_trace.shape
    in_maps, cfg = _host_prep(state_trace, w1a, b1a, Ta, w1b, b1b, Tb, NC)

    key = tuple(sorted(cfg.items()))
    if key not in _CACHE:
        _CACHE[key] = _build_nc(**cfg)
    nc = _CACHE[key]

    res = run_bass_kernel_spmd(nc, in_maps, core_ids=list(range(NC)))
    out = np.empty((B, D), np.float32)
    DL = D // NC
    for c in range(NC):
        out[:, c * DL:(c + 1) * DL] = res.results[c]["out"]
    return out


# revision 16
# speedup vs baseline: 1.0722x; 1.0722x over previous
"""Trainium2 Bass kernel for the per-feature grouped MLP (SuperLinear/GLU x2).

Math (per feature d of D=2048, batch B=512, M=32, H=64):
  x1 = state[:, d, :] @ w1a[:, :, d] / Ta + b1a[d]      [B, 128]
  h  = x1[:, :64] * sigmoid(x1[:, 64:])                 [B, 64]
  x2 = h @ w1b[:, :, d] / Tb + b1b[d]                   [B, 2]
  out[:, d] = x2[:, 0] * sigmoid(x2[:, 1])

Sharding: D split across 8 cores (embarrassingly parallel), 256 features/core.

Device dataflow per core (gen = 4 features, super-gen = 2 gens):
  MM1 (fp16): PE in 32x64 array-tiling mode. Feature j of a gen owns K-rows
  32j..32j+32. Even gens map feature j to array col-half (j%2), odd gens to
  (1-j%2), so the two gens of a super-gen use complementary array tiles and
  run concurrently (8 tiles busy). Bank layout: A = [a_even; a_odd] per
  feature-pair, G likewise => GLU operands are partition-aligned. A/G are
  [128, 1024] (2 PSUM banks, one pair per bank).
  GLU1: sigmoid (ACT) [128,1024] PSUM->SBUF fp32; tensor_mul (DVE)
  [128,1024] -> x2 (fp16, SBUF). For odd gens the pair rows are swapped
  (f_odd on top) — compensated in the host-built w2 quad weights.
  MM2 (fp16): x2 chunk [128,128] stationary (fp16 => FWL), rhs = block-diag
  pair weights [128, 4] -> quad columns [128b, 4] into a per-window PSUM
  bank at col bc*QR + pair_local*4. Burst every `burst` gens.
  GLU2 per 16-gen window: strided sigmoid over c1 cols + tensor_mul, one
  3D-AP output DMA.
"""

import numpy as np

_CACHE = {}


def _build_nc(B, DL, M, H, burst, window, use_ba, use_bg, use_bq):
    import concourse.bass as bass
    import concourse.mybir as mybir
    from concourse import bacc
    from concourse.tile import TileContext

    f32 = mybir.dt.float32
    f16 = mybir.dt.float16
    H2 = 2 * H
    NGEN = DL // 4  # gens of 4 features
    assert NGEN % window == 0 and window % burst == 0
    assert NGEN % 2 == 0 and burst % 2 == 0
    NB = B // 128  # b-chunks for MM2
    QR = 8 * window  # quad cols per b-chunk region (2*window pairs x 4)
    FW = 4 * window  # features (output cols) per window

    nc = bacc.Bacc("TRN2", target_bir_lowering=False)

    # st: [128=(j,m), NGEN*B]; w: [128=(j,m), NGEN*128=(gen,(wa|wg))]
    st_d = nc.dram_tensor("st", [128, NGEN * B], f16, kind="ExternalInput")
    w_d = nc.dram_tensor("w", [128, NGEN * H2], f16, kind="ExternalInput")
    # w2 quad weights, window-major: [nwin*128, 32*4]
    w2_d = nc.dram_tensor("w2", [(NGEN // window) * H2, 2 * window * 4], f16,
                          kind="ExternalInput")
    if use_bg:
        bg_d = nc.dram_tensor("bg", [DL, H], f32, kind="ExternalInput")
    if use_ba:
        ba_d = nc.dram_tensor("ba", [DL, H], f32, kind="ExternalInput")
    if use_bq:
        bq_d = nc.dram_tensor("bq", [DL // 2, 4], f32, kind="ExternalInput")
    out_d = nc.dram_tensor("out", [B, DL], f32, kind="ExternalOutput")

    Sig = mybir.ActivationFunctionType.Sigmoid
    Mult = mybir.AluOpType.mult

    with TileContext(nc) as tc:
        with tc.tile_pool(name="sb", bufs=4) as sb:
            x2_list = {}
            if use_bq:
                bq_t = sb.tile([1, DL * 2], f32, tag="bq", bufs=1, name="bqt")
                ones_t = sb.tile([1, 128], f16, tag="ones", bufs=1,
                                 name="onest")
                nc.sync.dma_start(out=bq_t,
                                  in_=bq_d.rearrange("p q -> 1 (p q)"))
                nc.vector.memset(ones_t, 1.0)

            # ================= Phase 1: MM1 + GLU1, x2 staged ==========
            with tc.tile_pool(name="ps1", bufs=4, space="PSUM") as ps:
                for sg in range(NGEN // 2):
                    g0 = 2 * sg
                    st_t = sb.tile([128, 2 * B], f16, tag="st", bufs=6,
                                   name=f"st{sg}")
                    nc.sync.dma_start(out=st_t,
                                      in_=st_d[:, g0 * B:(g0 + 2) * B])
                    w_t = sb.tile([128, 2 * H2], f16, tag="w", bufs=6,
                                  name=f"w{sg}")
                    nc.gpsimd.dma_start(out=w_t,
                                        in_=w_d[:, g0 * H2:(g0 + 2) * H2])

                    gens = []
                    for gi in range(2):
                        g = g0 + gi
                        G = ps.tile([128, 2 * B], f32, tag="mm1", bufs=4,
                                    name=f"G{g}")
                        A = ps.tile([128, 2 * B], f32, tag="mm1", bufs=4,
                                    name=f"A{g}")
                        gens.append((g, gi, A, G))

                    # MM1: 16 tiled matmuls, 2 rounds x 8 concurrent.
                    # gen parity gi: feature j -> array col-half (j%2)^gi.
                    for rnd in range(2):
                        for g, gi, A, G in gens:
                            for j in range(4):
                                rs = slice(32 * j, 32 * j + 32)
                                cp = 64 * ((j % 2) ^ gi)
                                fb = B * (j // 2)
                                do_a = (j % 2) == rnd
                                dst = A if do_a else G
                                wq = 0 if do_a else H
                                nc.tensor.matmul(
                                    out=dst[cp:cp + 64, fb:fb + B],
                                    lhsT=w_t[rs,
                                             gi * H2 + wq:gi * H2 + wq + H],
                                    rhs=st_t[rs, gi * B:(gi + 1) * B],
                                    start=True, stop=True,
                                    tile_position=(32 * j, cp))

                    # GLU1 per gen
                    for g, gi, A, G in gens:
                        sig = sb.tile([128, 2 * B], f32, tag="sig",
                                      name=f"sig{g}")
                        x2 = sb.tile([128, 2 * B], f16, tag="x2", bufs=NGEN,
                                     name=f"x2{g}")
                        if use_bg:
                            bg_t = sb.tile([128, 2], f32, tag="bg",
                                           name=f"bg{g}")
                            nc.sync.dma_start(
                                out=bg_t,
                                in_=bg_d[4 * g:4 * g + 4, :].rearrange(
                                    "(p two) h -> (two h) p", two=2))
                            for k in range(2):
                                nc.scalar.activation(
                                    out=sig[:, k * B:(k + 1) * B],
                                    in_=G[:, k * B:(k + 1) * B], func=Sig,
                                    bias=bg_t[:, k:k + 1])
                        else:
                            nc.scalar.activation(out=sig, in_=G, func=Sig)
                        if use_ba:
                            ba_t = sb.tile([128, 2], f32, tag="ba",
                                           name=f"ba{g}")
                            nc.sync.dma_start(
                                out=ba_t,
                                in_=ba_d[4 * g:4 * g + 4, :].rearrange(
                                    "(p two) h -> (two h) p", two=2))
                            for k in range(2):
                                nc.vector.scalar_tensor_tensor(
                                    out=x2[:, k * B:(k + 1) * B],
                                    in0=A[:, k * B:(k + 1) * B],
                                    scalar=ba_t[:, k:k + 1],
                                    in1=sig[:, k * B:(k + 1) * B],
                                    op0=mybir.AluOpType.add, op1=Mult)
                        else:
                            nc.vector.tensor_tensor(out=x2, in0=A, in1=sig,
                                                    op=Mult)
                        x2_list[g] = x2

            # ================= Phase 2: MM2 + GLU2 =====================
            with tc.tile_pool(name="ps2", bufs=4, space="PSUM") as ps2:
                for w in range(NGEN // window):
                    quad = ps2.tile([128, NB * QR], f32, tag="quad", bufs=4,
                                    name=f"quad{w}")
                    w2_t = sb.tile([H2, 2 * window * 4], f16, tag="w2",
                                   bufs=2, name=f"w2t{w}")
                    nc.gpsimd.dma_start(
                        out=w2_t, in_=w2_d[w * H2:(w + 1) * H2, :])
                    for gg in range(w * window, (w + 1) * window):
                        x2g = x2_list.pop(gg)
                        for p in range(2):
                            pl = (gg % window) * 2 + p  # pair idx in window
                            for bc in range(NB):
                                qo = bc * QR + pl * 4
                                nc.tensor.matmul(
                                    out=quad[:, qo:qo + 4],
                                    lhsT=x2g[:, p * B + bc * 128:
                                             p * B + (bc + 1) * 128],
                                    rhs=w2_t[:, pl * 4:pl * 4 + 4],
                                    start=True, stop=not use_bq)
                                if use_bq:
                                    pg = (gg * 2 + p)  # global pair
                                    nc.tensor.matmul(
                                        out=quad[:, qo:qo + 4],
                                        lhsT=ones_t,
                                        rhs=bq_t[:, pg * 4:pg * 4 + 4],
                                        start=False, stop=True)
                    sig2 = sb.tile([128, NB * QR // 2], f32, tag="sig2",
                                   bufs=2, name=f"sig2{w}")
                    o_t = sb.tile([128, NB * QR // 2], f32, tag="o", bufs=2,
                                  name=f"o{w}")
                    nc.scalar.activation(
                        out=sig2, in_=quad[:, 1:NB * QR:2], func=Sig)
                    nc.vector.tensor_tensor(
                        out=o_t, in0=quad[:, 0:NB * QR:2], in1=sig2, op=Mult)
                    dst = out_d.rearrange("(bc p) d -> p bc d", bc=NB)
                    nc.sync.dma_start(
                        out=dst[:, :, w * FW:(w + 1) * FW],
                        in_=o_t.rearrange("p (bc d) -> p bc d", bc=NB))
    nc.finalize()
    return nc


def _gen_major(a, NC, NGEN):
    """[D, 32, X] -> per-core [128=(j,m), NGEN*X] with gen-major free dim."""
    D = a.shape[0]
    X = a.shape[2]
    r = a.reshape(NC, NGEN, 4 * 32, X).transpose(0, 2, 1, 3)
    return np.ascontiguousarray(r.reshape(NC, 128, NGEN * X))


def _host_prep(state_trace, w1a, b1a, Ta, w1b, b1b, Tb, NC):
    import ml_dtypes  # noqa: F401  (fp16 is native numpy)

    B, D, M = state_trace.shape
    H2 = w1a.shape[1]
    H = H2 // 2
    DL = D // NC
    window = 16
    NGEN = DL // 4

    Ta_v = float(np.asarray(Ta).reshape(-1)[0])
    Tb_v = float(np.asarray(Tb).reshape(-1)[0])

    # state: [B, D, M] -> [D, M, B] fp16 -> gen-major
    st = np.asarray(state_trace, np.float32).transpose(1, 2, 0)
    st = _gen_major(st.astype(np.float16), NC, NGEN)

    # w1a: [M, 2H, D]/Ta -> [D, M, 2H] fp16 (cols: wa | wg) -> gen-major
    w1aT = (np.asarray(w1a, np.float32).transpose(2, 0, 1)
            * np.float32(1.0 / Ta_v))
    w = _gen_major(w1aT.astype(np.float16), NC, NGEN)

    # w2 block-diag quads: [D/2 pairs, 2H, 4], cols (c0f0,c1f0,c0f1,c1f1)
    # where f0 = even feature of the pair. For pairs of ODD gens the x2
    # partition blocks are swapped (f_odd on top), so swap the row blocks.
    w1bT = (np.asarray(w1b, np.float32).transpose(2, 0, 1)
            * np.float32(1.0 / Tb_v))  # [D, H, 2]
    w2q = np.zeros((D // 2, H2, 4), np.float32)
    pr = np.arange(D // 2)
    odd = (pr // 2) % 2 == 1  # pair's gen parity
    ev = ~odd
    w2q[ev, :H, 0] = w1bT[0::2][ev, :, 0]
    w2q[ev, :H, 1] = w1bT[0::2][ev, :, 1]
    w2q[ev, H:, 2] = w1bT[1::2][ev, :, 0]
    w2q[ev, H:, 3] = w1bT[1::2][ev, :, 1]
    w2q[odd, H:, 0] = w1bT[0::2][odd, :, 0]
    w2q[odd, H:, 1] = w1bT[0::2][odd, :, 1]
    w2q[odd, :H, 2] = w1bT[1::2][odd, :, 0]
    w2q[odd, :H, 3] = w1bT[1::2][odd, :, 1]
    nwin = NGEN // window
    w2q = w2q.reshape(NC, nwin, 2 * window, H2, 4).transpose(0, 1, 3, 2, 4)
    w2q = np.ascontiguousarray(
        w2q.reshape(NC, nwin * H2, 2 * window * 4)).astype(np.float16)

    # biases (device order: for odd gens the pair rows are swapped)
    b1a_f = np.asarray(b1a, np.float32).reshape(D, H2) * np.float32(1 / Ta_v)
    gperm = np.arange(D).reshape(-1, 4)
    gperm[1::2] = gperm[1::2][:, [1, 0, 3, 2]]
    gperm = gperm.reshape(-1)
    ba = np.ascontiguousarray(b1a_f[gperm, :H])
    bg = np.ascontiguousarray(b1a_f[gperm, H:])
    b1b_f = np.asarray(b1b, np.float32).reshape(D, 2) * np.float32(1 / Tb_v)
    bq = np.zeros((D // 2, 4), np.float32)
    bq[:, 0] = b1b_f[0::2, 0]
    bq[:, 1] = b1b_f[0::2, 1]
    bq[:, 2] = b1b_f[1::2, 0]
    bq[:, 3] = b1b_f[1::2, 1]

    use_ba = bool(np.any(ba))
    use_bg = bool(np.any(bg))
    use_bq = bool(np.any(bq))

    in_maps = []
    for c in range(NC):
        ds = slice(c * DL, (c + 1) * DL)
        m = {"st": st[c], "w": w[c], "w2": w2q[c]}
        if use_bg:
            m["bg"] = np.ascontiguousarray(bg[ds])
        if use_ba:
            m["ba"] = np.ascontiguousarray(ba[ds])
        if use_bq:
            m["bq"] = np.ascontiguousarray(bq[c * DL // 2:(c + 1) * DL // 2])
        in_maps.append(m)
    import os
    burst = int(os.environ.get("K_BURST", "4"))
    cfg = dict(B=B, DL=DL, M=M, H=H, burst=burst, window=window,
               use_ba=use_ba, use_bg=use_bg, use_bq=use_bq)
    return in_maps, cfg


def kernel(state_trace, w1a, b1a, Ta, w1b, b1b, Tb):
    from concourse.bass_utils import run_bass_kernel_spmd

    NC = 8
    B, D, M = state_trace.shape
    in_maps, cfg = _host_prep(state_trace, w1a, b1a, Ta, w1b, b1b, Tb, NC)

    key = tuple(sorted(cfg.items()))
    if key not in _CACHE:
        _CACHE[key] = _build_nc(**cfg)
    nc = _CACHE[key]

    res = run_bass_kernel_spmd(nc, in_maps, core_ids=list(range(NC)))
    out = np.empty((B, D), np.float32)
    DL = D // NC
    for c in range(NC):
        out[:, c * DL:(c + 1) * DL] = res.results[c]["out"]
    return out


# revision 18
# speedup vs baseline: 1.0910x; 1.0175x over previous
"""Trainium2 Bass kernel for the per-feature grouped MLP (SuperLinear/GLU x2).

Math (per feature d of D=2048, batch B=512, M=32, H=64):
  x1 = state[:, d, :] @ w1a[:, :, d] / Ta + b1a[d]      [B, 128]
  h  = x1[:, :64] * sigmoid(x1[:, 64:])                 [B, 64]
  x2 = h @ w1b[:, :, d] / Tb + b1b[d]                   [B, 2]
  out[:, d] = x2[:, 0] * sigmoid(x2[:, 1])

Sharding: D split across 8 cores (embarrassingly parallel), 256 features/core.

Device dataflow per core (gen = 4 features, super-gen = 2 gens):
  MM1 (fp16): PE in 32x64 array-tiling mode. Feature j of a gen owns K-rows
  32j..32j+32. Even gens map feature j to array col-half (j%2), odd gens to
  (1-j%2), so the two gens of a super-gen use complementary array tiles and
  run concurrently (8 tiles busy). Bank layout: A = [a_even; a_odd] per
  feature-pair, G likewise => GLU operands are partition-aligned. A/G are
  [128, 1024] (2 PSUM banks, one pair per bank).
  GLU1: sigmoid (ACT) [128,1024] PSUM->SBUF fp32; tensor_mul (DVE)
  [128,1024] -> x2 (fp16, SBUF). For odd gens the pair rows are swapped
  (f_odd on top) — compensated in the host-built w2 quad weights.
  MM2 (fp16): x2 chunk [128,128] stationary (fp16 => FWL), rhs = block-diag
  pair weights [128, 4] -> quad columns [128b, 4] into a per-window PSUM
  bank at col bc*QR + pair_local*4. Burst every `burst` gens.
  GLU2 per 16-gen window: strided sigmoid over c1 cols + tensor_mul, one
  3D-AP output DMA.
"""

import numpy as np

_CACHE = {}


def _build_nc(B, DL, M, H, burst, window, use_ba, use_bg, use_bq):
    import concourse.bass as bass
    import concourse.mybir as mybir
    from concourse import bacc
    from concourse.tile import TileContext

    f32 = mybir.dt.float32
    f16 = mybir.dt.float16
    H2 = 2 * H
    NGEN = DL // 4  # gens of 4 features
    assert NGEN % window == 0 and window % burst == 0
    assert NGEN % 2 == 0 and burst % 2 == 0
    NB = B // 128  # b-chunks for MM2
    QR = 8 * window  # quad cols per b-chunk region (2*window pairs x 4)
    FW = 4 * window  # features (output cols) per window

    nc = bacc.Bacc("TRN2", target_bir_lowering=False)

    # st: [128=(j,m), NGEN*B]; w: [128=(j,m), NGEN*128=(gen,(wa|wg))]
    st_d = nc.dram_tensor("st", [128, NGEN * B], f16, kind="ExternalInput")
    w_d = nc.dram_tensor("w", [128, NGEN * H2], f16, kind="ExternalInput")
    # w2 quad weights, window-major: [nwin*128, 32*4]
    w2_d = nc.dram_tensor("w2", [(NGEN // window) * H2, 2 * window * 4], f16,
                          kind="ExternalInput")
    if use_bg:
        bg_d = nc.dram_tensor("bg", [DL, H], f32, kind="ExternalInput")
    if use_ba:
        ba_d = nc.dram_tensor("ba", [DL, H], f32, kind="ExternalInput")
    if use_bq:
        bq_d = nc.dram_tensor("bq", [DL // 2, 4], f32, kind="ExternalInput")
    out_d = nc.dram_tensor("out", [B, DL], f32, kind="ExternalOutput")

    Sig = mybir.ActivationFunctionType.Sigmoid
    Mult = mybir.AluOpType.mult

    with TileContext(nc) as tc:
        with tc.tile_pool(name="sb", bufs=4) as sb:
            x2_list = {}
            if use_bq:
                bq_t = sb.tile([1, DL * 2], f32, tag="bq", bufs=1, name="bqt")
                ones_t = sb.tile([1, 128], f16, tag="ones", bufs=1,
                                 name="onest")
                nc.sync.dma_start(out=bq_t,
                                  in_=bq_d.rearrange("p q -> 1 (p q)"))
                nc.vector.memset(ones_t, 1.0)

            # warm up the sigmoid table while the first DMAs run
            warm = sb.tile([1, 8], f32, tag="warm", bufs=1, name="warm")
            nc.vector.memset(warm, 0.0)
            nc.scalar.activation(out=warm, in_=warm, func=Sig)

            # ================= Phase 1: MM1 + GLU1, x2 staged ==========
            with tc.tile_pool(name="ps1", bufs=4, space="PSUM") as ps:
                for sg in range(NGEN // 2):
                    g0 = 2 * sg
                    st_t = sb.tile([128, 2 * B], f16, tag="st", bufs=6,
                                   name=f"st{sg}")
                    nc.sync.dma_start(out=st_t,
                                      in_=st_d[:, g0 * B:(g0 + 2) * B])
                    w_t = sb.tile([128, 2 * H2], f16, tag="w", bufs=6,
                                  name=f"w{sg}")
                    nc.gpsimd.dma_start(out=w_t,
                                        in_=w_d[:, g0 * H2:(g0 + 2) * H2])

                    gens = []
                    for gi in range(2):
                        g = g0 + gi
                        G = ps.tile([128, 2 * B], f32, tag="mm1", bufs=4,
                                    name=f"G{g}")
                        A = ps.tile([128, 2 * B], f32, tag="mm1", bufs=4,
                                    name=f"A{g}")
                        gens.append((g, gi, A, G))

                    # MM1: 16 tiled matmuls, 2 rounds x 8 concurrent.
                    # gen parity gi: feature j -> array col-half (j%2)^gi.
                    for rnd in range(2):
                        for g, gi, A, G in gens:
                            for j in range(4):
                                rs = slice(32 * j, 32 * j + 32)
                                cp = 64 * ((j % 2) ^ gi)
                                fb = B * (j // 2)
                                do_a = (j % 2) == rnd
                                dst = A if do_a else G
                                wq = 0 if do_a else H
                                nc.tensor.matmul(
                                    out=dst[cp:cp + 64, fb:fb + B],
                                    lhsT=w_t[rs,
                                             gi * H2 + wq:gi * H2 + wq + H],
                                    rhs=st_t[rs, gi * B:(gi + 1) * B],
                                    start=True, stop=True,
                                    tile_position=(32 * j, cp))

                    # GLU1 per gen
                    for g, gi, A, G in gens:
                        sig = sb.tile([128, 2 * B], f32, tag="sig",
                                      name=f"sig{g}")
                        x2 = sb.tile([128, 2 * B], f16, tag="x2", bufs=NGEN,
                                     name=f"x2{g}")
                        if use_bg:
                            bg_t = sb.tile([128, 2], f32, tag="bg",
                                           name=f"bg{g}")
                            nc.sync.dma_start(
                                out=bg_t,
                                in_=bg_d[4 * g:4 * g + 4, :].rearrange(
                                    "(p two) h -> (two h) p", two=2))
                            for k in range(2):
                                nc.scalar.activation(
                                    out=sig[:, k * B:(k + 1) * B],
                                    in_=G[:, k * B:(k + 1) * B], func=Sig,
                                    bias=bg_t[:, k:k + 1])
                        else:
                            nc.scalar.activation(out=sig, in_=G, func=Sig)
                        if use_ba:
                            ba_t = sb.tile([128, 2], f32, tag="ba",
                                           name=f"ba{g}")
                            nc.sync.dma_start(
                                out=ba_t,
                                in_=ba_d[4 * g:4 * g + 4, :].rearrange(
                                    "(p two) h -> (two h) p", two=2))
                            for k in range(2):
                                nc.vector.scalar_tensor_tensor(
                                    out=x2[:, k * B:(k + 1) * B],
                                    in0=A[:, k * B:(k + 1) * B],
                                    scalar=ba_t[:, k:k + 1],
                                    in1=sig[:, k * B:(k + 1) * B],
                                    op0=mybir.AluOpType.add, op1=Mult)
                        else:
                            nc.vector.tensor_tensor(out=x2, in0=A, in1=sig,
                                                    op=Mult)
                        x2_list[g] = x2

                # ============= Phase 2: MM2 + GLU2 (same PSUM pool — no
                # pool barrier, so early windows overlap the phase-1 tail)
                for w in range(NGEN // window):
                    quadf = ps.tile([128, 2 * B], f32, tag="mm1", bufs=4,
                                    name=f"quad{w}")
                    quad = quadf[:, :NB * QR]
                    w2_t = sb.tile([H2, 2 * window * 4], f16, tag="w2",
                                   bufs=2, name=f"w2t{w}")
                    nc.gpsimd.dma_start(
                        out=w2_t, in_=w2_d[w * H2:(w + 1) * H2, :])
                    for gg in range(w * window, (w + 1) * window):
                        x2g = x2_list.pop(gg)
                        for p in range(2):
                            pl = (gg % window) * 2 + p  # pair idx in window
                            for bc in range(NB):
                                qo = bc * QR + pl * 4
                                nc.tensor.matmul(
                                    out=quad[:, qo:qo + 4],
                                    lhsT=x2g[:, p * B + bc * 128:
                                             p * B + (bc + 1) * 128],
                                    rhs=w2_t[:, pl * 4:pl * 4 + 4],
                                    start=True, stop=not use_bq)
                                if use_bq:
                                    pg = (gg * 2 + p)  # global pair
                                    nc.tensor.matmul(
                                        out=quad[:, qo:qo + 4],
                                        lhsT=ones_t,
                                        rhs=bq_t[:, pg * 4:pg * 4 + 4],
                                        start=False, stop=True)
                    sig2 = sb.tile([128, NB * QR // 2], f32, tag="sig2",
                                   bufs=2, name=f"sig2{w}")
                    o_t = sb.tile([128, NB * QR // 2], f32, tag="o", bufs=2,
                                  name=f"o{w}")
                    nc.scalar.activation(
                        out=sig2, in_=quad[:, 1:NB * QR:2], func=Sig)
                    nc.vector.tensor_tensor(
                        out=o_t, in0=quad[:, 0:NB * QR:2], in1=sig2, op=Mult)
                    dst = out_d.rearrange("(bc p) d -> p bc d", bc=NB)
                    nc.sync.dma_start(
                        out=dst[:, :, w * FW:(w + 1) * FW],
                        in_=o_t.rearrange("p (bc d) -> p bc d", bc=NB))
    nc.finalize()
    return nc


def _gen_major(a, NC, NGEN):
    """[D, 32, X] -> per-core [128=(j,m), NGEN*X] with gen-major free dim."""
    D = a.shape[0]
    X = a.shape[2]
    r = a.reshape(NC, NGEN, 4 * 32, X).transpose(0, 2, 1, 3)
    return np.ascontiguousarray(r.reshape(NC, 128, NGEN * X))


def _host_prep(state_trace, w1a, b1a, Ta, w1b, b1b, Tb, NC):
    import ml_dtypes  # noqa: F401  (fp16 is native numpy)

    B, D, M = state_trace.shape
    H2 = w1a.shape[1]
    H = H2 // 2
    DL = D // NC
    window = 16
    NGEN = DL // 4

    Ta_v = float(np.asarray(Ta).reshape(-1)[0])
    Tb_v = float(np.asarray(Tb).reshape(-1)[0])

    # state: [B, D, M] -> [D, M, B] fp16 -> gen-major
    st = np.asarray(state_trace, np.float32).transpose(1, 2, 0)
    st = _gen_major(st.astype(np.float16), NC, NGEN)

    # w1a: [M, 2H, D]/Ta -> [D, M, 2H] fp16 (cols: wa | wg) -> gen-major
    w1aT = (np.asarray(w1a, np.float32).transpose(2, 0, 1)
            * np.float32(1.0 / Ta_v))
    w = _gen_major(w1aT.astype(np.float16), NC, NGEN)

    # w2 block-diag quads: [D/2 pairs, 2H, 4], cols (c0f0,c1f0,c0f1,c1f1)
    # where f0 = even feature of the pair. For pairs of ODD gens the x2
    # partition blocks are swapped (f_odd on top), so swap the row blocks.
    w1bT = (np.asarray(w1b, np.float32).transpose(2, 0, 1)
            * np.float32(1.0 / Tb_v))  # [D, H, 2]
    w2q = np.zeros((D // 2, H2, 4), np.float32)
    pr = np.arange(D // 2)
    odd = (pr // 2) % 2 == 1  # pair's gen parity
    ev = ~odd
    w2q[ev, :H, 0] = w1bT[0::2][ev, :, 0]
    w2q[ev, :H, 1] = w1bT[0::2][ev, :, 1]
    w2q[ev, H:, 2] = w1bT[1::2][ev, :, 0]
    w2q[ev, H:, 3] = w1bT[1::2][ev, :, 1]
    w2q[odd, H:, 0] = w1bT[0::2][odd, :, 0]
    w2q[odd, H:, 1] = w1bT[0::2][odd, :, 1]
    w2q[odd, :H, 2] = w1bT[1::2][odd, :, 0]
    w2q[odd, :H, 3] = w1bT[1::2][odd, :, 1]
    nwin = NGEN // window
    w2q = w2q.reshape(NC, nwin, 2 * window, H2, 4).transpose(0, 1, 3, 2, 4)
    w2q = np.ascontiguousarray(
        w2q.reshape(NC, nwin * H2, 2 * window * 4)).astype(np.float16)

    # biases (device order: for odd gens the pair rows are swapped)
    b1a_f = np.asarray(b1a, np.float32).reshape(D, H2) * np.float32(1 / Ta_v)
    gperm = np.arange(D).reshape(-1, 4)
    gperm[1::2] = gperm[1::2][:, [1, 0, 3, 2]]
    gperm = gperm.reshape(-1)
    ba = np.ascontiguousarray(b1a_f[gperm, :H])
    bg = np.ascontiguousarray(b1a_f[gperm, H:])
    b1b_f = np.asarray(b1b, np.float32).reshape(D, 2) * np.float32(1 / Tb_v)
    bq = np.zeros((D // 2, 4), np.float32)
    bq[:, 0] = b1b_f[0::2, 0]
    bq[:, 1] = b1b_f[0::2, 1]
    bq[:, 2] = b1b_f[1::2, 0]
    bq[:, 3] = b1b_f[1::2, 1]

    use_ba = bool(np.any(ba))
    use_bg = bool(np.any(bg))
    use_bq = bool(np.any(bq))

    in_maps = []
    for c in range(NC):
        ds = slice(c * DL, (c + 1) * DL)
        m = {"st": st[c], "w": w[c], "w2": w2q[c]}
        if use_bg:
            m["bg"] = np.ascontiguousarray(bg[ds])
        if use_ba:
            m["ba"] = np.ascontiguousarray(ba[ds])
        if use_bq:
            m["bq"] = np.ascontiguousarray(bq[c * DL // 2:(c + 1) * DL // 2])
        in_maps.append(m)
    import os
    burst = int(os.environ.get("K_BURST", "4"))
    cfg = dict(B=B, DL=DL, M=M, H=H, burst=burst, window=window,
               use_ba=use_ba, use_bg=use_bg, use_bq=use_bq)
    return in_maps, cfg


def kernel(state_trace, w1a, b1a, Ta, w1b, b1b, Tb):
    from concourse.bass_utils import run_bass_kernel_spmd

    NC = 8
    B, D, M = state_trace.shape
    in_maps, cfg = _host_prep(state_trace, w1a, b1a, Ta, w1b, b1b, Tb, NC)

    key = tuple(sorted(cfg.items()))
    if key not in _CACHE:
        _CACHE[key] = _build_nc(**cfg)
    nc = _CACHE[key]

    res = run_bass_kernel_spmd(nc, in_maps, core_ids=list(range(NC)))
    out = np.empty((B, D), np.float32)
    DL = D // NC
    for c in range(NC):
        out[:, c * DL:(c + 1) * DL] = res.results[c]["out"]
    return out
